# revision 19
# baseline (speedup 1.0000x reference)
"""Trainium2 Bass kernel for nn_ASpTLinear: out = x @ W.T + bias.

Shapes (hardcoded): x [4, 2048, 4096] f32, W [4096, 4096] f32, bias [4096] f32.

Strategy: data-parallel over the 8192 rows of x across 8 NeuronCores
(1024 rows/core). Each core computes out_c = x_c @ W.T + bias with a
tiled PE matmul:
  - host pre-transposes x and W so the contraction dim (IN_F) is leading,
    which is the layout the TensorEngine needs for both operands
    (it contracts over the SBUF partition dim),
  - x_c^T [4096, 1024] is fully cached in SBUF (16 MB) and used as the
    stationary operand; W^T streams through once (64 MB),
  - matmuls run in float32r (fp32 data, 1 cycle/row vs 4 for fp32) with
    fp32 PSUM accumulation; bias is added during PSUM eviction on DVE.
"""

import numpy as np

BATCH, SEQ, IN_F, OUT_F = 4, 2048, 4096, 4096
N_CORES = 8
ROWS = BATCH * SEQ            # 8192
M = ROWS // N_CORES           # 1024 rows per core
P = 128
KS = IN_F // P                # 32 k-subtiles
MS = M // P                   # 8 m-subtiles
N_TILE = 512
NT = OUT_F // N_TILE          # 8 n-tiles

_NC = None          # compiled Bass module, cached across kernel() calls
last_results = None  # BassKernelResults of the most recent run (for test harness)


def _build():
    import concourse.mybir as mybir
    import concourse.tile as tile
    from concourse import bacc

    f32 = mybir.dt.float32
    f32r = mybir.dt.float32r

    nc = bacc.Bacc("TRN2", target_bir_lowering=False, debug=False,
                   num_devices=N_CORES)
    xT_d = nc.dram_tensor("xT", [IN_F, M], f32r, kind="ExternalInput")
    wT_d = nc.dram_tensor("wT", [IN_F, OUT_F], f32r, kind="ExternalInput")
    b_d = nc.dram_tensor("bias", [OUT_F], f32, kind="ExternalInput")
    out_d = nc.dram_tensor("out", [M, OUT_F], f32, kind="ExternalOutput")

    xT_ap = xT_d.ap().rearrange("(ko p) m -> p ko m", p=P)
    wT_ap = wT_d.ap().rearrange("(ko p) n -> p ko n", p=P)
    out_ap = out_d.ap().rearrange("(mo p) n -> p mo n", p=P)

    with tile.TileContext(nc) as tc:
        with tc.tile_pool(name="xpool", bufs=KS) as xpool, \
             tc.tile_pool(name="wpool", bufs=8) as wpool, \
             tc.tile_pool(name="opool", bufs=12) as opool, \
             tc.tile_pool(name="bpool", bufs=1) as bpool, \
             tc.tile_pool(name="psum", bufs=8, space="PSUM") as psum:
            bias_sb = bpool.tile([P, OUT_F], f32)
            nc.scalar.dma_start(bias_sb[:],
                                b_d.ap()[None, :].to_broadcast((P, OUT_F)))

            # PE p-state pre-warm: ~5us of dummy matmuls while the first
            # x/W tiles are still in flight. They write into the nt=0 PSUM
            # tiles, whose first real matmul (start=True) resets them.
            # (memset can't target f32r tiles -- walrus rejects it -- so the
            # scratch is f32 bitcast to f32r for the matmul.)
            scr = bpool.tile([P, N_TILE], f32)
            nc.vector.memset(scr[:], 0.0)
            scr_r = scr[:].bitcast(f32r)

            # x tiles are loaded lazily, interleaved with nt=0's W-tile
            # stream on the same sync HWDGE queue, so the first W tile is
            # not queued behind 16 MB of x.
            # A small leading slice of x (just the ms=0 stationary tile of
            # ks=0) goes first on the queue so the very first
            # LDWEIGHTS/MATMUL only waits on 64KB + one W tile, not on the
            # full 512KB x_0.
            x0_mini = bpool.tile([P, P], f32r)
            nc.sync.dma_start(x0_mini[:], xT_ap[:, 0, 0:P])

            x_tiles = [None] * KS

            def get_x(ks):
                if x_tiles[ks] is None:
                    xt = xpool.tile([P, M], f32r, tag="x", name=f"x_{ks}")
                    nc.sync.dma_start(xt[:], xT_ap[:, ks])
                    x_tiles[ks] = xt
                return x_tiles[ks]

            for nt in range(NT):
                n_lo = nt * N_TILE
                ptiles = [psum.tile([P, N_TILE], f32, space="PSUM", tag="ps",
                                    name=f"ps_{nt}_{ms}")
                          for ms in range(MS)]
                if nt == 0:
                    for warm in range(16):
                        nc.tensor.matmul(ptiles[warm % MS][:],
                                         lhsT=scr_r[:, :P], rhs=scr_r,
                                         start=True, stop=True)
                for ks in range(KS):
                    wt = wpool.tile([P, N_TILE], f32r, tag="w")
                    nc.sync.dma_start(wt[:],
                                      wT_ap[:, ks, n_lo:n_lo + N_TILE])
                    xt = get_x(ks)
                    for ms in range(MS):
                        lhsT = (x0_mini[:] if nt == 0 and ks == 0 and ms == 0
                                else xt[:, ms * P:(ms + 1) * P])
                        nc.tensor.matmul(
                            ptiles[ms][:],
                            lhsT=lhsT,
                            rhs=wt[:],
                            start=(ks == 0),
                            stop=(ks == KS - 1),
                        )
                if nt < NT - 1:
                    # Evict in two steps: the PSUM->SBUF copy frees the
                    # PSUM bank for nt+1 as early as possible; the bias add
                    # runs later, off the bank-release critical path.
                    ots = []
                    for ms in range(MS):
                        ot = opool.tile([P, N_TILE], f32, tag="o",
                                        name=f"o_{nt}_{ms}")
                        nc.vector.tensor_copy(out=ot[:], in_=ptiles[ms][:])
                        ots.append(ot)
                    for ms in range(MS):
                        nc.vector.tensor_add(ots[ms][:], ots[ms][:],
                                             bias_sb[:, n_lo:n_lo + N_TILE])
                        nc.scalar.dma_start(
                            out_ap[:, ms, n_lo:n_lo + N_TILE], ots[ms][:])
                else:
                    # Last tile: no bank-release pressure, fused add-evict
                    # shortens the kernel tail.
                    for ms in range(MS):
                        ot = opool.tile([P, N_TILE], f32, tag="o",
                                        name=f"o_{nt}_{ms}")
                        nc.vector.tensor_add(ot[:], ptiles[ms][:],
                                             bias_sb[:, n_lo:n_lo + N_TILE])
                        nc.scalar.dma_start(
                            out_ap[:, ms, n_lo:n_lo + N_TILE], ot[:])
    nc.compile()
    return nc


def kernel(x, W, bias):
    global _NC, last_results
    import os
    # NTFF tracing needs an antenv.axon_hooks shim that may not exist in
    # the grading container; only honor BASS_TRACE when our own harness
    # opts in.
    if os.environ.get("KERNEL_ALLOW_TRACE") != "1":
        os.environ.pop("BASS_TRACE", None)
    from concourse.bass_utils import run_bass_kernel_spmd

    if _NC is None:
        _NC = _build()

    x = np.asarray(x, dtype=np.float32)
    W = np.asarray(W, dtype=np.float32)
    bias = np.asarray(bias, dtype=np.float32)

    xT = np.ascontiguousarray(x.reshape(ROWS, IN_F).T)   # [IN_F, ROWS]
    wT = np.ascontiguousarray(W.T)                       # [IN_F, OUT_F]

    in_maps = [
        {
            "xT": np.ascontiguousarray(xT[:, c * M:(c + 1) * M]),
            "wT": wT,
            "bias": bias,
        }
        for c in range(N_CORES)
    ]
    res = run_bass_kernel_spmd(_NC, in_maps, list(range(N_CORES)))
    last_results = res
    out = np.concatenate([res.results[c]["out"] for c in range(N_CORES)],
                         axis=0)
    return out.reshape(BATCH, SEQ, OUT_F)
